# revision 23
# baseline (speedup 1.0000x reference)
"""Trainium2 Bass kernel for nn_AdjacencyProcessing (8192x8192 adjacency
normalisation), distributed row-wise across 8 NeuronCores.

out[i,j] = s_i * A[i,j] + d_i * eye[i,j]
  rs_i = sum_j A[i,j]
  s_i  = 1 / (max(1, rs_i) * (rs_i + 1))
  d_i  = (1 + REG) / (rs_i + 1)

Each core processes a [1024, 8192] row shard: row sums and row scaling are
fully local; the eye addition targets the local diagonal block, whose column
offset comes from partition_id() at runtime (SPMD-uniform program). I/O is
bf16 (well within the accuracy budget for uniform [0,1) data) which halves
HBM traffic; compute is fp32 internally.
"""
import numpy as np

N = 8192
NCORES = 8
ROWS = N // NCORES  # 1024 rows per core
P = 128             # SBUF partitions
NT = ROWS // P      # 8 tiles per core
REG = 0.001

_cached_nc = None


def _build():
    import concourse.bass as bass
    import concourse.bacc as bacc
    import concourse.mybir as mybir
    from concourse.tile import TileContext

    nc = bacc.Bacc("TRN2", target_bir_lowering=False, debug=False,
                   num_devices=NCORES)
    adj = nc.declare_dram_parameter("adjacency", [ROWS, N], mybir.dt.bfloat16,
                                    isOutput=False)
    eye = nc.declare_dram_parameter("eye", [P, P], mybir.dt.bfloat16,
                                    isOutput=False)
    out = nc.declare_dram_parameter("out", [ROWS, N], mybir.dt.bfloat16,
                                    isOutput=True)
    with TileContext(nc) as tc:
        with tc.tile_pool(name="data", bufs=1) as pool, \
             tc.tile_pool(name="small", bufs=2 * NT) as spool, \
             tc.tile_pool(name="eyep", bufs=1) as eyep:
            eyet = eyep.tile([P, P], mybir.dt.bfloat16)
            nc.sync.dma_start(out=eyet[:], in_=eye[:, :])
            # one contiguous SBUF block; tile i is the column slice
            # [i*N, (i+1)*N) so adjacent tiles can be stored as one DMA
            block = pool.tile([P, NT * N], mybir.dt.bfloat16)
            tiles = [block[:, i * N:(i + 1) * N] for i in range(NT)]
            # Phase 1: prefetch every tile on the SP HWDGE ring. No load ever
            # waits, and the later stores queue strictly behind the loads in
            # the same FIFO.
            for i in range(NT):
                nc.sync.dma_start(out=tiles[i], in_=adj[i * P:(i + 1) * P, :])
            # Phase 2: per-tile compute. Row sums alternate ACT
            # (copy+accumulate) / DVE (tensor_reduce) so reduce throughput
            # matches load arrival; the small chain and the bf16 4x-mode
            # scale run on DVE.
            pid = nc.vector.partition_id()
            for i in range(NT):
                tile = tiles[i]
                rs = spool.tile([P, 1], mybir.dt.float32, tag="rs")
                nc.scalar.activation(tile[:], tile[:],
                                     mybir.ActivationFunctionType.Copy,
                                     scale=1.0, accum_out=rs[:])
                m = spool.tile([P, 1], mybir.dt.float32, tag="m")
                nc.vector.tensor_scalar_max(m[:], rs[:], 1.0)
                denom = spool.tile([P, 1], mybir.dt.float32, tag="denom")
                nc.vector.tensor_scalar_add(denom[:], rs[:], 1.0)
                prod = spool.tile([P, 1], mybir.dt.float32, tag="prod")
                nc.vector.tensor_mul(prod[:], m[:], denom[:])
                s = spool.tile([P, 1], mybir.dt.float32, tag="s")
                nc.vector.reciprocal(s[:], prod[:])
                dn = spool.tile([P, 1], mybir.dt.float32, tag="dn")
                nc.vector.reciprocal(dn[:], denom[:])
                d = spool.tile([P, 1], mybir.dt.float32, tag="d")
                nc.vector.tensor_scalar_mul(d[:], dn[:], 1.0 + REG)
                # scale rows in place on DVE (bf16 tensor_scalar hits 4x mode)
                nc.vector.tensor_scalar_mul(tile[:], tile[:], s[:])
                # diagonal: add d*eye into the local diagonal block, at the
                # runtime column offset (pid*NT + i) * P
                eyed = spool.tile([P, P], mybir.dt.bfloat16, tag="eyed")
                nc.vector.tensor_scalar_mul(eyed[:], eyet[:], d[:])
                dyn = bass.ts(pid * NT + i, P)
                nc.vector.tensor_add(tile[:, dyn], tile[:, dyn], eyed[:])
            # Phase 3: stores, also on the SP ring — FIFO-ordered behind all
            # loads; by the time the ring reaches store i, its data is ready.
            # Adjacent tiles are contiguous in SBUF and in DRAM, so pair them
            # into 4MB ops; the last two stay single so their readiness still
            # beats the ring.
            for j in range(NT // 2 - 1):
                dview = out[2 * j * P:(2 * j + 2) * P, :].rearrange(
                    "(t p) m -> p t m", p=P)
                sview = block[:, 2 * j * N:(2 * j + 2) * N].rearrange(
                    "p (t m) -> p t m", t=2)
                nc.sync.dma_start(out=dview, in_=sview)
            for i in (NT - 2, NT - 1):
                nc.sync.dma_start(out=out[i * P:(i + 1) * P, :],
                                  in_=tiles[i])
    nc.finalize()
    return nc


def run(adjacency: np.ndarray, trace: bool = False):
    """Run on 8 NeuronCores; returns (full_out, BassKernelResults)."""
    global _cached_nc
    import concourse.mybir as mybir
    from concourse.bass_utils import run_bass_kernel_spmd

    bf16 = mybir.dt.np(mybir.dt.bfloat16)
    adjacency = np.asarray(adjacency)
    assert adjacency.shape == (N, N)
    adj_bf16 = np.ascontiguousarray(adjacency.astype(bf16))
    eye = np.eye(P, dtype=bf16)
    if _cached_nc is None:
        _cached_nc = _build()
    in_maps = [{"adjacency": adj_bf16[c * ROWS:(c + 1) * ROWS], "eye": eye}
               for c in range(NCORES)]
    res = run_bass_kernel_spmd(_cached_nc, in_maps,
                               core_ids=list(range(NCORES)), trace=trace)
    full = np.empty((N, N), dtype=np.float32)
    for c in range(NCORES):
        full[c * ROWS:(c + 1) * ROWS] = res.results[c]["out"]
    return full, res


def _run_in_subprocess(adjacency: np.ndarray) -> np.ndarray:
    """Fallback for transient NRT 'exec unit unrecoverable' faults, which are
    sticky within a process: rerun in a fresh interpreter/NRT session."""
    import os
    import subprocess
    import sys
    import tempfile

    with tempfile.TemporaryDirectory() as td:
        inp = os.path.join(td, "in.npy")
        outp = os.path.join(td, "out.npy")
        np.save(inp, np.ascontiguousarray(np.asarray(adjacency,
                                                     dtype=np.float32)))
        code = (
            "import numpy as np, importlib.util\n"
            f"spec = importlib.util.spec_from_file_location('kmod', {__file__!r})\n"
            "m = importlib.util.module_from_spec(spec)\n"
            "spec.loader.exec_module(m)\n"
            f"a = np.load({inp!r})\n"
            "o, _ = m.run(a, trace=False)\n"
            f"np.save({outp!r}, o)\n"
        )
        err = b""
        for _ in range(2):
            r = subprocess.run([sys.executable, "-c", code],
                               capture_output=True)
            if r.returncode == 0 and os.path.exists(outp):
                return np.load(outp)
            err = r.stderr
        raise RuntimeError(f"subprocess kernel failed: {err[-2000:]!r}")


def kernel(adjacency: np.ndarray) -> np.ndarray:
    try:
        out, _ = run(adjacency, trace=False)
        return out
    except Exception:
        return _run_in_subprocess(adjacency)
